# revision 7
# baseline (speedup 1.0000x reference)
"""GatedGCN forward on 8 Trainium2 NeuronCores (Bass kernel), with a NumPy
fallback if the hardware path is unavailable.

Strategy: nodes are sharded contiguously across the 8 cores; edges are
assigned to the core owning their destination. Each message-passing step
AllGathers the per-node messages (fp16, node-major), then every core
computes its destinations' segment-sums as indicator matmuls over
dma_gather'ed source rows. GraphNorm statistics are one-hot matmuls +
AllReduce. All heavy lifting (matmuls, GRU, GELU) runs on-device in fp16
with fp32 PSUM accumulation.

All expensive setup (jax/axon init, Bass build, NEFF compile, device
warm-up) happens at import; kernel() only preprocesses edges on CPU,
uploads, runs, and downloads.
"""
import sys
import numpy as np

N, E, C, IN, OUT, L, G = 20000, 320000, 256, 128, 128, 3, 16
EPS = 1e-5
_CORES = 8

sys.path.insert(0, "/opt/trn_rl_repo")

_HW = None          # (cfg, runner) when hardware path is ready
_HW_ERR = None


# ----------------------------------------------------------------------
# NumPy fallback (scipy-accelerated) — always available, always correct.
# ----------------------------------------------------------------------

def _kernel_numpy(x, edge_index, batch, gcn_w, gcn_b, gn0_w, gn0_b, gn0_ms,
                  ggc_w, gru_wih, gru_whh, gru_bih, gru_bhh,
                  gn_w, gn_b, gn_ms, lin_w, lin_b):
    from scipy import sparse as sp
    from scipy.special import erf

    def seg_rows(v, idx, n):
        out = np.zeros((n,) + v.shape[1:], np.float32)
        np.add.at(out, idx, v)
        return out

    x = np.asarray(x, np.float32)
    edge_index = np.asarray(edge_index, np.int64)
    batch = np.asarray(batch, np.int64)
    n = x.shape[0]
    loop = np.arange(n)
    row = np.concatenate([edge_index[0], loop])
    col = np.concatenate([edge_index[1], loop])
    deg = np.bincount(col, minlength=n).astype(np.float32)
    dinv = 1.0 / np.sqrt(np.maximum(deg, 1.0))
    enorm = (dinv[row] * dinv[col]).astype(np.float32)
    A_gcn = sp.coo_matrix((enorm, (col, row)), shape=(n, n), dtype=np.float32).tocsr()
    src, dst = edge_index[0], edge_index[1]
    A_msg = sp.coo_matrix((np.ones(src.shape[0], np.float32), (dst, src)),
                          shape=(n, n), dtype=np.float32).tocsr()
    cnt = np.maximum(np.bincount(batch, minlength=G), 1.0).astype(np.float32)[:, None]

    def graph_norm(xx, w, b, ms):
        mean = seg_rows(xx, batch, G) / cnt
        out = xx - mean[batch] * ms
        var = seg_rows(out * out, batch, G) / cnt
        return w * out / np.sqrt(var + EPS)[batch] + b

    def gru(a, h, wih, whh, bih, bhh):
        gi = a @ wih.T + bih
        gh = h @ whh.T + bhh
        ir, iz, i_n = np.split(gi, 3, -1)
        hr, hz, h_n = np.split(gh, 3, -1)
        r = 1.0 / (1.0 + np.exp(-(ir + hr)))
        z = 1.0 / (1.0 + np.exp(-(iz + hz)))
        nn = np.tanh(i_n + r * h_n)
        return (1.0 - z) * nn + z * h

    h = np.asarray(A_gcn @ (x @ np.asarray(gcn_w, np.float32).T), np.float32) + gcn_b
    h = graph_norm(h, gn0_w, gn0_b, gn0_ms)
    for l in range(L):
        g = h
        for i in range(2):
            a = np.asarray(A_msg @ (g @ np.asarray(ggc_w[l, i], np.float32)), np.float32)
            g = gru(a, g, gru_wih[l], gru_whh[l], gru_bih[l], gru_bhh[l])
        g = g * 0.5 * (1.0 + erf(g / np.sqrt(2.0)))
        h = h + graph_norm(g.astype(np.float32), gn_w[l], gn_b[l], gn_ms[l])
    return (h @ np.asarray(lin_w, np.float32).T + lin_b).astype(np.float32)


# ----------------------------------------------------------------------
# Hardware path
# ----------------------------------------------------------------------

def _hw_init():
    global _HW, _HW_ERR
    try:
        import jax
        try:
            cur = jax.config.jax_platforms
        except AttributeError:
            cur = None
        if not cur or "axon" not in cur:
            # keep CPU as the default backend so host-side jax code in the
            # caller's process is unaffected; axon devices stay addressable.
            jax.config.update("jax_platforms", "cpu,axon")
        cfg = _Cfg()
        nc = _build(cfg)
        r = _ShardedRunner(nc, _CORES)
        from concurrent.futures import ThreadPoolExecutor
        ex = ThreadPoolExecutor(1)
        try:
            ex.submit(r.warmup).result(timeout=900)
        finally:
            ex.shutdown(wait=False)
        _HW = (cfg, r)
    except Exception as e:  # noqa: BLE001
        import traceback
        _HW_ERR = traceback.format_exc()
        _HW = None


def kernel(**inputs):
    inputs = {k: np.asarray(v) for k, v in inputs.items()}
    if _HW is not None:
        try:
            from concurrent.futures import ThreadPoolExecutor
            ex = ThreadPoolExecutor(1)
            try:
                return ex.submit(_kernel_hw, **inputs).result(timeout=120)
            finally:
                ex.shutdown(wait=False)
        except Exception:
            pass
    return _kernel_numpy(**inputs)


def _kernel_hw(x, edge_index, batch, **weights):
    import jax
    from concurrent.futures import ThreadPoolExecutor
    cfg, r = _HW
    x = np.asarray(x, np.float32)
    edge_index = np.asarray(edge_index, np.int64)
    batch = np.asarray(batch, np.int64)
    pool = ThreadPoolExecutor(16)
    try:
        def do_fast():
            fast = _pre_fast(cfg, x, batch, weights)
            return {n: r.put_shards(pool, v) for n, v in fast.items()}
        fast_fut = pool.submit(do_fast)
        edges = _pre_edges(cfg, edge_index)
        edge_futs = {n: r.put_shards(pool, v) for n, v in edges.items()}
        all_futs = {**fast_fut.result(), **edge_futs}
        arrs = [r.assemble(all_futs[n]) for n in r.in_names]
        outs = r.run(arrs)
        jax.block_until_ready(outs)
        res = r.fetch(outs)
    finally:
        pool.shutdown(wait=False)
    return res[0].reshape(cfg.n, cfg.cout).astype(np.float32)


# ======================================================================
# Bass kernel builder (see module docstring for the design)
# ======================================================================

def _make_cfg_and_build():
    import concourse.bass as bass  # noqa: F401
    import concourse.bacc as bacc
    import concourse.mybir as mybir
    import concourse.tile as tile
    return bacc, mybir, tile


class _Cfg:
    def __init__(self, cores=_CORES, n=N, c=C, cin=IN, cout=OUT, L_=L, g=G, e=E):
        self.cores, self.n, self.c, self.cin, self.cout = cores, n, c, cin, cout
        self.L, self.g, self.e = L_, g, e
        self.h3 = 3 * c
        self.npc = n // cores
        self.nb = (self.npc + 127) // 128
        self.nbp = [min(128, self.npc - 128 * b) for b in range(self.nb)]

        def chunks(mean):
            return int(np.ceil((mean + 6.0 * np.sqrt(mean)) / 128.0))
        self.kc = [chunks(e * w / n) for w in self.nbp]
        self.kg = [chunks((e + n) * w / n) for w in self.nbp]
        self.kmax = max(max(self.kc), max(self.kg))
        self.nchm = sum(self.kc)
        self.nchg = sum(self.kg)
        self.idxwm = self.nchm * 8
        self.idxwg = self.nchg * 8
        self.cb = c // 128
        self.tiles = []
        off = 0
        while off < self.npc:
            tl = min(512, self.npc - off)
            self.tiles.append((off, tl))
            off += tl
        off = 0
        self.w16 = {}

        def add16(name, *shape):
            nonlocal off
            self.w16[name] = (off, shape)
            off += int(np.prod(shape))
        add16("gcnwT", cin, c)
        for l in range(L_):
            for i in range(2):
                add16(f"W_{l}_{i}", c, c)
            add16(f"wihT_{l}", c, self.h3)
            add16(f"whhT_{l}", c, self.h3)
        add16("linwT", c, cout)
        add16("gcnb", 1, c)
        add16("linb", 1, cout)
        self.wt16 = ((off + 8 * 256 - 1) // (8 * 256)) * (8 * 256)
        off = 0
        self.w32 = {}

        def add32(name, *shape):
            nonlocal off
            self.w32[name] = (off, shape)
            off += int(np.prod(shape))
        for l in range(L_):
            add32(f"brz_{l}", 128, 4)
            add32(f"bin_{l}", 128, self.cb)
            add32(f"bhn_{l}", 128, self.cb)
        for t in ["0"] + [str(l + 1) for l in range(L_)]:
            add32(f"msrep_{t}", g, c)
            add32(f"msfac_{t}", g, c)
            add32(f"wrep_{t}", g, c)
            add32(f"bcol_{t}", 128, self.cb)
        add32("invcnt", g, 1)
        self.wt32 = ((off + 8 * 256 - 1) // (8 * 256)) * (8 * 256)


def _build(cfg):
    bacc, mybir, tile = _make_cfg_and_build()
    f16, f32, i16 = mybir.dt.float16, mybir.dt.float32, mybir.dt.int16
    AF = mybir.ActivationFunctionType
    ALU = mybir.AluOpType

    nc = bacc.Bacc("TRN2", target_bir_lowering=False, debug=False,
                   enable_asserts=False, num_devices=cfg.cores)
    C_, CB, NB, NPC, G_ = cfg.c, cfg.cb, cfg.nb, cfg.npc, cfg.g
    RG = [list(range(cfg.cores))]
    SH = "Shared" if cfg.cores > 4 else "Local"

    xT_d = nc.dram_tensor("xT", [cfg.cin, NPC], f16, kind="ExternalInput")
    w16_d = nc.dram_tensor("w16", [cfg.wt16 // cfg.cores], f16, kind="ExternalInput")
    w32_d = nc.dram_tensor("w32", [cfg.wt32 // cfg.cores], f32, kind="ExternalInput")
    idxm_d = nc.dram_tensor("idxm", [16, cfg.idxwm], i16, kind="ExternalInput")
    idxg_d = nc.dram_tensor("idxg", [16, cfg.idxwg], i16, kind="ExternalInput")
    dstm_d = nc.dram_tensor("dstm", [128, cfg.nchm], f32, kind="ExternalInput")
    dstg_d = nc.dram_tensor("dstg", [128, cfg.nchg], f32, kind="ExternalInput")
    eno_d = nc.dram_tensor("eno", [128, cfg.nchg], f16, kind="ExternalInput")
    ohnm_d = nc.dram_tensor("ohnm", [128, NB * G_], f16, kind="ExternalInput")
    ohT_d = nc.dram_tensor("ohT", [G_, NPC], f16, kind="ExternalInput")
    out_d = nc.dram_tensor("out", [NPC, cfg.cout], f16, kind="ExternalOutput")

    kcoff = np.concatenate([[0], np.cumsum(cfg.kc)]).astype(int)
    kgoff = np.concatenate([[0], np.cumsum(cfg.kg)]).astype(int)
    bslice = [slice(128 * b, 128 * b + cfg.nbp[b]) for b in range(NB)]

    with tile.TileContext(nc) as tc:
        with (
            tc.tile_pool(name="cst", bufs=1) as cst,
            tc.tile_pool(name="st", bufs=1) as st,
            tc.tile_pool(name="wk", bufs=2) as wk,
            tc.tile_pool(name="gt", bufs=2) as gt,
            tc.tile_pool(name="psA", bufs=2, space="PSUM") as psA,
            tc.tile_pool(name="psB", bufs=1, space="PSUM") as psB,
            tc.tile_pool(name="dr", bufs=1, space="DRAM") as dr,
        ):
            wsh16 = dr.tile([cfg.wt16 // cfg.cores], f16)
            wfull16 = dr.tile([cfg.wt16], f16, addr_space=SH)
            nc.sync.dma_start(wsh16[:], w16_d[:])
            nc.gpsimd.collective_compute(
                "AllGather", ALU.bypass, replica_groups=RG,
                ins=[wsh16.opt()], outs=[wfull16.opt()])
            wsh32 = dr.tile([cfg.wt32 // cfg.cores], f32)
            wfull32 = dr.tile([cfg.wt32], f32, addr_space=SH)
            nc.sync.dma_start(wsh32[:], w32_d[:])
            nc.gpsimd.collective_compute(
                "AllGather", ALU.bypass, replica_groups=RG,
                ins=[wsh32.opt()], outs=[wfull32.opt()])

            def w16ap(name):
                off, shape = cfg.w16[name]
                return wfull16[off:off + int(np.prod(shape))]

            def w32ap(name):
                off, shape = cfg.w32[name]
                return wfull32[off:off + int(np.prod(shape))]

            gcnwT = cst.tile([128, C_], f16)
            nc.sync.dma_start(gcnwT[:], w16ap("gcnwT").rearrange("(p c) -> p c", p=128))
            Wsb, wihT, whhT = [], [], []
            for l in range(cfg.L):
                row = []
                for i in range(2):
                    t = cst.tile([128, CB, C_], f16, name=f"Wsb_{l}_{i}")
                    nc.sync.dma_start(t[:], w16ap(f"W_{l}_{i}").rearrange(
                        "(j p c) -> p j c", p=128, j=CB))
                    row.append(t)
                Wsb.append(row)
                t = cst.tile([128, CB, cfg.h3], f16, name=f"wihT_{l}")
                nc.sync.dma_start(t[:], w16ap(f"wihT_{l}").rearrange(
                    "(j p c) -> p j c", p=128, j=CB))
                wihT.append(t)
                t = cst.tile([128, CB, cfg.h3], f16, name=f"whhT_{l}")
                nc.sync.dma_start(t[:], w16ap(f"whhT_{l}").rearrange(
                    "(j p c) -> p j c", p=128, j=CB))
                whhT.append(t)
            linwT = cst.tile([128, CB, cfg.cout], f16)
            nc.sync.dma_start(linwT[:], w16ap("linwT").rearrange(
                "(j p c) -> p j c", p=128, j=CB))
            gcnb16 = cst.tile([1, C_], f16)
            nc.sync.dma_start(gcnb16[:], w16ap("gcnb").rearrange("(o c) -> o c", o=1))
            linb16 = cst.tile([1, cfg.cout], f16)
            nc.sync.dma_start(linb16[:], w16ap("linb").rearrange("(o c) -> o c", o=1))

            brz, bin_, bhn = [], [], []
            for l in range(cfg.L):
                t = cst.tile([128, 4], f32, name=f"brz{l}")
                nc.sync.dma_start(t[:], w32ap(f"brz_{l}").rearrange("(p c) -> p c", p=128))
                brz.append(t)
                t = cst.tile([128, CB], f32, name=f"bin{l}")
                nc.sync.dma_start(t[:], w32ap(f"bin_{l}").rearrange("(p c) -> p c", p=128))
                bin_.append(t)
                t = cst.tile([128, CB], f32, name=f"bhn{l}")
                nc.sync.dma_start(t[:], w32ap(f"bhn_{l}").rearrange("(p c) -> p c", p=128))
                bhn.append(t)
            gnw = {}
            for tname in ["0"] + [str(l + 1) for l in range(cfg.L)]:
                for part in ["msrep", "msfac", "wrep"]:
                    t = cst.tile([G_, C_], f32, name=f"{part}{tname}")
                    nc.sync.dma_start(t[:], w32ap(f"{part}_{tname}").rearrange(
                        "(p c) -> p c", p=G_))
                    gnw[f"{part}_{tname}"] = t
                t = cst.tile([128, CB], f32, name=f"bcol{tname}")
                nc.sync.dma_start(t[:], w32ap(f"bcol_{tname}").rearrange(
                    "(p c) -> p c", p=128))
                gnw[f"bcol_{tname}"] = t
            invcnt = cst.tile([G_, 1], f32)
            nc.sync.dma_start(invcnt[:], w32ap("invcnt").rearrange("(p c) -> p c", p=G_))

            iota_row = cst.tile([128, 128], f32)
            nc.gpsimd.iota(iota_row[:], pattern=[[1, 128]], base=0,
                           channel_multiplier=0, allow_small_or_imprecise_dtypes=True)
            iota_col = cst.tile([128, 1], f32)
            nc.gpsimd.iota(iota_col[:], pattern=[[0, 1]], base=0,
                           channel_multiplier=1, allow_small_or_imprecise_dtypes=True)
            ident = cst.tile([128, 128], f16)
            nc.vector.tensor_scalar(ident[:], iota_row[:], iota_col[:], None,
                                    ALU.is_equal)
            ones_row = cst.tile([1, 128], f16)
            nc.vector.memset(ones_row[:], 1.0)
            epscol = cst.tile([128, 1], f32)
            nc.vector.memset(epscol[:], EPS)

            def load_idx(tile_sb, dram_t):
                nc.sync.dma_start(tile_sb[0:16, :], dram_t[:, :])
                nc.gpsimd.dma_start(tile_sb[16:32, :], tile_sb[0:16, :])
                nc.gpsimd.dma_start(tile_sb[32:64, :], tile_sb[0:32, :])
                nc.gpsimd.dma_start(tile_sb[64:128, :], tile_sb[0:64, :])

            idxm = st.tile([128, cfg.idxwm], i16)
            load_idx(idxm, idxm_d)
            dstm = st.tile([128, cfg.nchm], f32)
            nc.sync.dma_start(dstm[:], dstm_d[:, :])
            ohnm = st.tile([128, NB, G_], f16)
            nc.sync.dma_start(ohnm[:], ohnm_d.ap().rearrange("p (b g) -> p b g", g=G_))
            ohT = st.tile([G_, NPC], f16)
            nc.sync.dma_start(ohT[:], ohT_d[:, :])

            hT = st.tile([128, CB, NPC], f32)
            gTf = st.tile([128, CB, NPC], f32)
            gT16 = st.tile([128, CB, NPC], f16)
            aT16 = st.tile([128, CB, NPC], f16)
            anm16 = st.tile([128, NB, C_], f16)

            def seg_sum(idx_sb, dst_sb, kk, koff, mfull, eno_sb, bias_row, pfx):
                for b in range(NB):
                    K = kk[b]
                    nbp = cfg.nbp[b]
                    gat = gt.tile([128, cfg.kmax, C_], f16, tag="gat",
                                  name=f"gat{pfx}_{b}")
                    nc.gpsimd.dma_gather(
                        out_ap=gat[:, 0:K, :],
                        in_ap=mfull[:],
                        idxs_ap=idx_sb[:, koff[b] * 8:(koff[b] + K) * 8],
                        num_idxs=K * 128,
                        num_idxs_reg=K * 128,
                        elem_size=C_,
                        single_packet=False,
                    )
                    ind = gt.tile([128, cfg.kmax, 128], f16, tag="ind",
                                  name=f"ind{pfx}_{b}")
                    iota_b = iota_row[:, 0:128].unsqueeze(1).broadcast_to((128, K, 128))
                    dst_b = dst_sb[:, koff[b]:koff[b] + K].unsqueeze(2).broadcast_to(
                        (128, K, 128))
                    nc.vector.tensor_tensor(ind[:, 0:K, :], iota_b, dst_b, ALU.is_equal)
                    if eno_sb is not None:
                        eno_b = eno_sb[:, koff[b]:koff[b] + K].unsqueeze(2).broadcast_to(
                            (128, K, 128))
                        nc.vector.tensor_tensor(ind[:, 0:K, :], ind[:, 0:K, :], eno_b,
                                                ALU.mult)
                    pa = psA.tile([128, 2, C_], f32, tag="pa", name=f"pa{pfx}_{b}")
                    for ch in range(K):
                        nc.tensor.matmul(pa[0:nbp, 0, :], ind[:, ch, 0:nbp],
                                         gat[:, ch, :], start=(ch == 0),
                                         stop=(ch == K - 1 and bias_row is None))
                    if bias_row is not None:
                        nc.tensor.matmul(pa[0:nbp, 0, :], ones_row[:, 0:nbp],
                                         bias_row[:], start=False, stop=True)
                    nc.vector.tensor_copy(anm16[0:nbp, b, :], pa[0:nbp, 0, :])

            def transpose_to_T(dst_T, src_nm, pfx):
                for b in range(NB):
                    nbp = cfg.nbp[b]
                    for j in range(CB):
                        pt = psA.tile([128, 2, C_], f16, tag="pmt",
                                      name=f"pt{pfx}_{b}_{j}")
                        nc.tensor.transpose(pt[:, 0, 0:nbp],
                                            src_nm[0:nbp, b, 128 * j:128 * (j + 1)],
                                            ident[0:nbp, 0:nbp])
                        nc.vector.tensor_copy(dst_T[:, j, bslice[b]], pt[:, 0, 0:nbp])

            def transpose_to_nm(dst_nm, src_T, pfx):
                for b in range(NB):
                    nbp = cfg.nbp[b]
                    for j in range(CB):
                        pt = psA.tile([128, 2, C_], f16, tag="pmt",
                                      name=f"ptn{pfx}_{b}_{j}")
                        nc.tensor.transpose(pt[0:nbp, 0, 0:128], src_T[:, j, bslice[b]],
                                            ident[:])
                        nc.vector.tensor_copy(dst_nm[0:nbp, b, 128 * j:128 * (j + 1)],
                                              pt[0:nbp, 0, 0:128])

            def graph_norm(tag, ynm, yT, dst, residual):
                ps1 = psB.tile([G_, 512], f32, tag="pr", name=f"ps1{tag}")
                ps2 = psB.tile([G_, 512], f32, tag="pz", name=f"ps2{tag}")
                for b in range(NB):
                    nbp = cfg.nbp[b]
                    ysq = wk.tile([128, C_], f16, tag="ysq", name=f"ysq{tag}_{b}")
                    nc.scalar.activation(ysq[0:nbp, :], ynm[0:nbp, b, :], AF.Square)
                    nc.tensor.matmul(ps1[:, 0:C_], ohnm[0:nbp, b, :], ynm[0:nbp, b, :],
                                     start=(b == 0), stop=(b == NB - 1))
                    nc.tensor.matmul(ps2[:, 0:C_], ohnm[0:nbp, b, :], ysq[0:nbp, :],
                                     start=(b == 0), stop=(b == NB - 1))
                s12 = wk.tile([G_, 2, C_], f32, tag="s12", name=f"s12{tag}")
                nc.vector.tensor_copy(s12[:, 0, :], ps1[:, 0:C_])
                nc.vector.tensor_copy(s12[:, 1, :], ps2[:, 0:C_])
                arin = dr.tile([G_, 2 * C_], f32, tag="arin", name=f"arin{tag}")
                arout = dr.tile([G_, 2 * C_], f32, tag="arout", name=f"arout{tag}",
                                addr_space=SH)
                nc.sync.dma_start(arin[:], s12[:].rearrange("p a c -> p (a c)"))
                nc.gpsimd.collective_compute(
                    "AllReduce", ALU.add, replica_groups=RG,
                    ins=[arin.opt()], outs=[arout.opt()])
                sm = wk.tile([G_, 2, C_], f32, tag="s12", name=f"sm{tag}")
                nc.sync.dma_start(sm[:].rearrange("p a c -> p (a c)"), arout[:])
                mean = wk.tile([G_, C_], f32, tag="gn1", name=f"mean{tag}")
                nc.vector.tensor_scalar(mean[:], sm[:, 0, :], invcnt[:], None, ALU.mult)
                ey2 = wk.tile([G_, C_], f32, tag="gn2", name=f"ey2{tag}")
                nc.vector.tensor_scalar(ey2[:], sm[:, 1, :], invcnt[:], None, ALU.mult)
                var = wk.tile([G_, C_], f32, tag="gn3", name=f"var{tag}")
                nc.vector.tensor_tensor(var[:], mean[:], mean[:], ALU.mult)
                nc.vector.tensor_tensor(var[:], var[:], gnw[f"msfac_{tag}"][:], ALU.mult)
                nc.vector.tensor_tensor(var[:], ey2[:], var[:], ALU.subtract)
                std = wk.tile([G_, C_], f32, tag="gn2", name=f"std{tag}")
                nc.scalar.activation(std[:], var[:], AF.Sqrt, bias=epscol[0:G_, :])
                istd = wk.tile([G_, C_], f32, tag="gn3", name=f"istd{tag}")
                nc.vector.reciprocal(istd[:], std[:])
                A16 = wk.tile([G_, C_], f16, tag="gnA", name=f"A16{tag}")
                nc.vector.tensor_tensor(A16[:], mean[:], gnw[f"msrep_{tag}"][:], ALU.mult)
                B16 = wk.tile([G_, C_], f16, tag="gnB", name=f"B16{tag}")
                nc.vector.tensor_tensor(B16[:], istd[:], gnw[f"wrep_{tag}"][:], ALU.mult)
                bcol = gnw[f"bcol_{tag}"]
                for j in range(CB):
                    for (toff, tl) in cfg.tiles:
                        pA = psB.tile([128, 512], f32, tag="pin",
                                      name=f"pA{tag}_{j}_{toff}")
                        pB = psB.tile([128, 512], f32, tag="phn",
                                      name=f"pB{tag}_{j}_{toff}")
                        nc.tensor.matmul(pA[:, 0:tl], A16[:, 128 * j:128 * (j + 1)],
                                         ohT[:, toff:toff + tl], start=True, stop=True)
                        nc.tensor.matmul(pB[:, 0:tl], B16[:, 128 * j:128 * (j + 1)],
                                         ohT[:, toff:toff + tl], start=True, stop=True)
                        t1 = wk.tile([128, 512], f32, tag="ew1",
                                     name=f"t1{tag}_{j}_{toff}")
                        nc.vector.tensor_tensor(t1[:, 0:tl], yT[:, j, toff:toff + tl],
                                                pA[:, 0:tl], ALU.subtract)
                        nc.vector.tensor_tensor(t1[:, 0:tl], t1[:, 0:tl], pB[:, 0:tl],
                                                ALU.mult)
                        nc.vector.tensor_scalar(t1[:, 0:tl], t1[:, 0:tl],
                                                bcol[:, j:j + 1], None, ALU.add)
                        if residual:
                            nc.vector.tensor_tensor(dst[:, j, toff:toff + tl],
                                                    dst[:, j, toff:toff + tl],
                                                    t1[:, 0:tl], ALU.add)
                        else:
                            nc.vector.tensor_copy(dst[:, j, toff:toff + tl],
                                                  t1[:, 0:tl])

            # ---- GCN ----
            with tc.tile_pool(name="gcn", bufs=1) as gp:
                xt = gp.tile([128, NPC], f16)
                nc.sync.dma_start(xt[:], xT_d[:, :])
                idxg = gp.tile([128, cfg.idxwg], i16)
                load_idx(idxg, idxg_d)
                dstg = gp.tile([128, cfg.nchg], f32)
                nc.sync.dma_start(dstg[:], dstg_d[:, :])
                eno = gp.tile([128, cfg.nchg], f16)
                nc.sync.dma_start(eno[:], eno_d[:, :])

                bnc = dr.tile([NPC, C_], f16, tag="bnc", name="bnc_gcn")
                mfull = dr.tile([cfg.n, C_], f16, tag="mfull", name="mfull_gcn",
                                addr_space=SH)
                for b in range(NB):
                    nbp = cfg.nbp[b]
                    pm = psA.tile([128, 2, C_], f32, tag="pmt", name=f"pxw{b}")
                    nc.tensor.matmul(pm[0:nbp, 0, :], xt[:, bslice[b]], gcnwT[:],
                                     start=True, stop=True)
                    mt = wk.tile([128, C_], f16, tag="mt", name=f"mxw{b}")
                    nc.vector.tensor_copy(mt[0:nbp, :], pm[0:nbp, 0, :])
                    nc.sync.dma_start(bnc[bslice[b], :], mt[0:nbp, :])
                nc.gpsimd.collective_compute(
                    "AllGather", ALU.bypass, replica_groups=RG,
                    ins=[bnc.opt()], outs=[mfull.opt()])
                seg_sum(idxg, dstg, cfg.kg, kgoff, mfull, eno, gcnb16, "g")
            transpose_to_T(aT16, anm16, "g")
            graph_norm("0", anm16, aT16, hT, residual=False)
            for j in range(CB):
                nc.vector.tensor_copy(gTf[:, j, :], hT[:, j, :])
                nc.vector.tensor_copy(gT16[:, j, :], hT[:, j, :])

            # ---- gated layers ----
            for l in range(cfg.L):
                for i in range(2):
                    sid = f"{l}_{i}"
                    bnc = dr.tile([NPC, C_], f16, tag="bnc", name=f"bnc{sid}")
                    mfull = dr.tile([cfg.n, C_], f16, tag="mfull", name=f"mfull{sid}",
                                    addr_space=SH)
                    for b in range(NB):
                        nbp = cfg.nbp[b]
                        pm = psA.tile([128, 2, C_], f32, tag="pmt", name=f"pm{sid}_{b}")
                        for j in range(CB):
                            nc.tensor.matmul(pm[0:nbp, 0, :], gT16[:, j, bslice[b]],
                                             Wsb[l][i][:, j, :], start=(j == 0),
                                             stop=(j == CB - 1))
                        mt = wk.tile([128, C_], f16, tag="mt", name=f"mm{sid}_{b}")
                        nc.vector.tensor_copy(mt[0:nbp, :], pm[0:nbp, 0, :])
                        nc.sync.dma_start(bnc[bslice[b], :], mt[0:nbp, :])
                    nc.gpsimd.collective_compute(
                        "AllGather", ALU.bypass, replica_groups=RG,
                        ins=[bnc.opt()], outs=[mfull.opt()])
                    seg_sum(idxm, dstm, cfg.kc, kcoff, mfull, None, None, sid)
                    transpose_to_T(aT16, anm16, sid)
                    for (toff, tl) in cfg.tiles:
                        tsl = slice(toff, toff + tl)
                        gnew = wk.tile([128, CB, 512], f32, tag="gnew",
                                       name=f"gnew{sid}_{toff}")
                        for cc in range(CB):
                            gid = f"{sid}_{toff}_{cc}"
                            pr = psB.tile([128, 512], f32, tag="pr", name=f"pr{gid}")
                            pz = psB.tile([128, 512], f32, tag="pz", name=f"pz{gid}")
                            pin = psB.tile([128, 512], f32, tag="pin", name=f"pi{gid}")
                            phn = psB.tile([128, 512], f32, tag="phn", name=f"ph{gid}")
                            for j in range(CB):
                                cr = cc * 128
                                cz = C_ + cc * 128
                                cn = 2 * C_ + cc * 128
                                nc.tensor.matmul(pr[:, 0:tl], wihT[l][:, j, cr:cr + 128],
                                                 aT16[:, j, tsl], start=(j == 0),
                                                 stop=False)
                                nc.tensor.matmul(pz[:, 0:tl], wihT[l][:, j, cz:cz + 128],
                                                 aT16[:, j, tsl], start=(j == 0),
                                                 stop=False)
                                nc.tensor.matmul(pin[:, 0:tl], wihT[l][:, j, cn:cn + 128],
                                                 aT16[:, j, tsl], start=(j == 0),
                                                 stop=(j == CB - 1))
                                nc.tensor.matmul(pr[:, 0:tl], whhT[l][:, j, cr:cr + 128],
                                                 gT16[:, j, tsl], start=False,
                                                 stop=(j == CB - 1))
                                nc.tensor.matmul(pz[:, 0:tl], whhT[l][:, j, cz:cz + 128],
                                                 gT16[:, j, tsl], start=False,
                                                 stop=(j == CB - 1))
                                nc.tensor.matmul(phn[:, 0:tl], whhT[l][:, j, cn:cn + 128],
                                                 gT16[:, j, tsl], start=(j == 0),
                                                 stop=(j == CB - 1))
                            r = wk.tile([128, 512], f32, tag="ew1", name=f"r{gid}")
                            nc.scalar.activation(r[:, 0:tl], pr[:, 0:tl], AF.Sigmoid,
                                                 bias=brz[l][:, cc:cc + 1])
                            z = wk.tile([128, 512], f32, tag="ew2", name=f"z{gid}")
                            nc.scalar.activation(z[:, 0:tl], pz[:, 0:tl], AF.Sigmoid,
                                                 bias=brz[l][:, 2 + cc:3 + cc])
                            hb = wk.tile([128, 512], f32, tag="ew3", name=f"hb{gid}")
                            nc.vector.tensor_scalar(hb[:, 0:tl], phn[:, 0:tl],
                                                    bhn[l][:, cc:cc + 1], None, ALU.add)
                            nc.vector.tensor_tensor(hb[:, 0:tl], r[:, 0:tl],
                                                    hb[:, 0:tl], ALU.mult)
                            nc.vector.tensor_tensor(hb[:, 0:tl], pin[:, 0:tl],
                                                    hb[:, 0:tl], ALU.add)
                            n_t = wk.tile([128, 512], f32, tag="ew1", name=f"n{gid}")
                            nc.scalar.activation(n_t[:, 0:tl], hb[:, 0:tl], AF.Tanh,
                                                 bias=bin_[l][:, cc:cc + 1])
                            d = wk.tile([128, 512], f32, tag="ew3", name=f"d{gid}")
                            nc.vector.tensor_tensor(d[:, 0:tl], gTf[:, cc, tsl],
                                                    n_t[:, 0:tl], ALU.subtract)
                            nc.vector.tensor_tensor(d[:, 0:tl], z[:, 0:tl],
                                                    d[:, 0:tl], ALU.mult)
                            nc.vector.tensor_tensor(gnew[:, cc, 0:tl], n_t[:, 0:tl],
                                                    d[:, 0:tl], ALU.add)
                        for cc in range(CB):
                            nc.vector.tensor_copy(gTf[:, cc, tsl], gnew[:, cc, 0:tl])
                            nc.vector.tensor_copy(gT16[:, cc, tsl], gnew[:, cc, 0:tl])
                for j in range(CB):
                    nc.scalar.activation(aT16[:, j, :], gTf[:, j, :], AF.Gelu)
                transpose_to_nm(anm16, aT16, f"y{l}")
                graph_norm(str(l + 1), anm16, aT16, hT, residual=True)
                if l < cfg.L - 1:
                    for j in range(CB):
                        nc.vector.tensor_copy(gTf[:, j, :], hT[:, j, :])
                        nc.vector.tensor_copy(gT16[:, j, :], hT[:, j, :])

            # ---- final linear ----
            for b in range(NB):
                nbp = cfg.nbp[b]
                h16 = wk.tile([128, CB, 128], f16, tag="h16", name=f"h16_{b}")
                for j in range(CB):
                    nc.vector.tensor_copy(h16[:, j, 0:nbp], hT[:, j, bslice[b]])
                po = psA.tile([128, 2, C_], f32, tag="pa", name=f"po{b}")
                for j in range(CB):
                    nc.tensor.matmul(po[0:nbp, 0, 0:cfg.cout], h16[:, j, 0:nbp],
                                     linwT[:, j, :], start=(j == 0), stop=False)
                nc.tensor.matmul(po[0:nbp, 0, 0:cfg.cout], ones_row[:, 0:nbp],
                                 linb16[:], start=False, stop=True)
                o16 = wk.tile([128, cfg.cout], f16, tag="mt", name=f"o16_{b}")
                nc.vector.tensor_copy(o16[0:nbp, :], po[0:nbp, 0, 0:cfg.cout])
                nc.sync.dma_start(out_d[bslice[b], :], o16[0:nbp, :])

    nc.compile()
    return nc


# ======================================================================
# CPU preprocessing
# ======================================================================

def _wrap_idx(idx):
    return np.ascontiguousarray(idx.reshape(-1, 16).T)


def _pre_fast(cfg, x, batch, w):
    """Input-edge-independent tensors: name -> list of per-core arrays."""
    C_, G_, L_ = cfg.c, cfg.g, cfg.L
    NPC, NB = cfg.npc, cfg.nb
    w16 = np.zeros(cfg.wt16, np.float16)

    def put16(name, arr):
        off, shape = cfg.w16[name]
        a = np.asarray(arr, np.float16).reshape(-1)
        w16[off:off + a.size] = a
    put16("gcnwT", np.asarray(w["gcn_w"]).T)
    for l in range(L_):
        for i in range(2):
            put16(f"W_{l}_{i}", w["ggc_w"][l, i])
        put16(f"wihT_{l}", np.asarray(w["gru_wih"][l]).T)
        put16(f"whhT_{l}", np.asarray(w["gru_whh"][l]).T)
    put16("linwT", np.asarray(w["lin_w"]).T)
    put16("gcnb", w["gcn_b"])
    put16("linb", w["lin_b"])

    w32 = np.zeros(cfg.wt32, np.float32)

    def put32(name, arr):
        off, shape = cfg.w32[name]
        a = np.asarray(arr, np.float32).reshape(-1)
        w32[off:off + a.size] = a
    for l in range(L_):
        bsum = np.asarray(w["gru_bih"][l]) + np.asarray(w["gru_bhh"][l])
        brz_cols = np.stack(
            [bsum[cc * 128:(cc + 1) * 128] for cc in range(cfg.cb)] +
            [bsum[C_ + cc * 128:C_ + (cc + 1) * 128] for cc in range(cfg.cb)],
            axis=1)
        put32(f"brz_{l}", brz_cols)
        put32(f"bin_{l}", np.asarray(w["gru_bih"][l])[2 * C_:].reshape(cfg.cb, 128).T)
        put32(f"bhn_{l}", np.asarray(w["gru_bhh"][l])[2 * C_:].reshape(cfg.cb, 128).T)
    gn_sets = {"0": (w["gn0_w"], w["gn0_b"], w["gn0_ms"])}
    for l in range(L_):
        gn_sets[str(l + 1)] = (w["gn_w"][l], w["gn_b"][l], w["gn_ms"][l])
    for tname, (wv, bv, msv) in gn_sets.items():
        msv = np.asarray(msv, np.float32)
        put32(f"msrep_{tname}", np.tile(msv[None, :], (G_, 1)))
        put32(f"msfac_{tname}", np.tile((2 * msv - msv * msv)[None, :], (G_, 1)))
        put32(f"wrep_{tname}", np.tile(np.asarray(wv, np.float32)[None, :], (G_, 1)))
        put32(f"bcol_{tname}", np.asarray(bv, np.float32).reshape(cfg.cb, 128).T)
    cnt = np.bincount(batch, minlength=G_).astype(np.float32)
    put32("invcnt", (1.0 / np.maximum(cnt, 1.0))[:, None])

    ohnm = np.zeros((cfg.cores, 128, NB * G_), np.float16)
    ohT = np.zeros((cfg.cores, G_, NPC), np.float16)
    for c in range(cfg.cores):
        bs = batch[c * NPC:(c + 1) * NPC]
        for b in range(NB):
            wd = cfg.nbp[b]
            gs = bs[128 * b:128 * b + wd]
            ohnm[c, np.arange(wd), b * G_ + gs] = 1.0
        ohT[c, bs, np.arange(NPC)] = 1.0

    xT = np.ascontiguousarray(np.asarray(x).T.astype(np.float16))
    w16s = w16.reshape(cfg.cores, -1)
    w32s = w32.reshape(cfg.cores, -1)
    return {
        "xT": [np.ascontiguousarray(xT[:, c * NPC:(c + 1) * NPC])
               for c in range(cfg.cores)],
        "w16": [w16s[c] for c in range(cfg.cores)],
        "w32": [w32s[c] for c in range(cfg.cores)],
        "ohnm": [ohnm[c] for c in range(cfg.cores)],
        "ohT": [ohT[c] for c in range(cfg.cores)],
    }


def _pre_edges(cfg, edge_index):
    N_, NPC, NB = cfg.n, cfg.npc, cfg.nb
    src = edge_index[0]
    dst = edge_index[1]
    deg = np.bincount(dst, minlength=N_).astype(np.float32) + 1.0
    dinv = 1.0 / np.sqrt(deg)

    def build_edges(src_a, dst_a, vals, kk):
        koff = np.concatenate([[0], np.cumsum(kk)]).astype(np.int64)
        nch = int(koff[-1])
        core = dst_a // NPC
        blk = (dst_a % NPC) // 128
        dstoff_v = (dst_a % NPC) % 128
        key = core * NB + blk
        order = np.argsort(key, kind="stable")
        so, kb, dv = src_a[order], key[order], dstoff_v[order]
        sv = vals[order] if vals is not None else None
        counts = np.bincount(key, minlength=cfg.cores * NB)
        cap = np.tile(np.array([kk[b] * 128 for b in range(NB)]), cfg.cores)
        if np.any(counts > cap):
            raise OverflowError("edge block overflow")
        starts = np.concatenate([[0], np.cumsum(counts)])[:-1]
        slot = np.arange(len(so)) - starts[kb]
        c_of = kb // NB
        b_of = kb % NB
        pos = koff[b_of] * 128 + slot
        idx_all = np.zeros((cfg.cores, nch * 128), np.int16)
        dstoff_all = np.full((cfg.cores, nch * 128), 128.0, np.float32)
        idx_all[c_of, pos] = so.astype(np.int16)
        dstoff_all[c_of, pos] = dv.astype(np.float32)
        if vals is not None:
            vals_all = np.zeros((cfg.cores, nch * 128), np.float16)
            vals_all[c_of, pos] = sv.astype(np.float16)
            vals_l = np.ascontiguousarray(
                vals_all.reshape(cfg.cores, nch, 128).transpose(0, 2, 1))
        else:
            vals_l = None
        idx_w = np.stack([_wrap_idx(idx_all[c]) for c in range(cfg.cores)])
        dstoff_l = np.ascontiguousarray(
            dstoff_all.reshape(cfg.cores, nch, 128).transpose(0, 2, 1))
        return idx_w, dstoff_l, vals_l

    idxm_w, dstm_l, _ = build_edges(src, dst, None, cfg.kc)
    loops = np.arange(N_, dtype=np.int64)
    g_src = np.concatenate([src, loops])
    g_dst = np.concatenate([dst, loops])
    g_vals = (dinv[g_src] * dinv[g_dst]).astype(np.float32)
    idxg_w, dstg_l, eno_l = build_edges(g_src, g_dst, g_vals, cfg.kg)
    return {
        "idxm": [idxm_w[c] for c in range(cfg.cores)],
        "idxg": [idxg_w[c] for c in range(cfg.cores)],
        "dstm": [dstm_l[c] for c in range(cfg.cores)],
        "dstg": [dstg_l[c] for c in range(cfg.cores)],
        "eno": [eno_l[c] for c in range(cfg.cores)],
    }


# ======================================================================
# Cached-jit SPMD runner (axon/PJRT)
# ======================================================================

class _ShardedRunner:
    def __init__(self, nc, n_cores):
        import jax
        import concourse.mybir as mybir
        from jax.sharding import Mesh, PartitionSpec, NamedSharding
        from jax.experimental.shard_map import shard_map
        from concourse.bass2jax import (_bass_exec_p, install_neuronx_cc_hook,
                                        partition_id_tensor)
        install_neuronx_cc_hook()
        self.jax = jax
        self.nc = nc
        self.n_cores = n_cores

        partition_name = (nc.partition_id_tensor.name
                          if nc.partition_id_tensor else None)
        in_names, out_names, out_avals, zero_shapes = [], [], [], []
        for alloc in nc.m.functions[0].allocations:
            if not isinstance(alloc, mybir.MemoryLocationSet):
                continue
            name = alloc.memorylocations[0].name
            if alloc.kind == "ExternalInput":
                if name != partition_name:
                    in_names.append(name)
            elif alloc.kind == "ExternalOutput":
                out_names.append(name)
                shape = tuple(alloc.tensor_shape)
                dtype = mybir.dt.np(alloc.dtype)
                out_avals.append(jax.core.ShapedArray(shape, dtype))
                zero_shapes.append((shape, dtype))
        self.in_names = list(in_names)
        self.out_names = list(out_names)
        self.out_avals = out_avals
        self._in_shapes = {}
        for alloc in nc.m.functions[0].allocations:
            if (isinstance(alloc, mybir.MemoryLocationSet)
                    and alloc.kind == "ExternalInput"):
                nm = alloc.memorylocations[0].name
                self._in_shapes[nm] = (tuple(alloc.tensor_shape),
                                       mybir.dt.np(alloc.dtype))
        n_params = len(in_names)
        n_outs = len(out_names)
        all_in_names = in_names + out_names
        if partition_name is not None:
            all_in_names.append(partition_name)
        nc_ref = nc

        def _body(*args):
            operands = list(args)
            if partition_name is not None:
                operands.append(partition_id_tensor())
            outs = _bass_exec_p.bind(
                *operands,
                out_avals=tuple(out_avals),
                in_names=tuple(all_in_names),
                out_names=tuple(out_names),
                lowering_input_output_aliases=(),
                sim_require_finite=False,
                sim_require_nnan=False,
                nc=nc_ref,
            )
            return tuple(outs)

        devices = jax.devices('axon')[:n_cores]
        self.devices = devices
        self.mesh = Mesh(np.asarray(devices), ("core",))
        in_specs = (PartitionSpec("core"),) * (n_params + n_outs)
        out_specs = (PartitionSpec("core"),) * n_outs
        donate = tuple(range(n_params, n_params + n_outs))
        self.sharding = NamedSharding(self.mesh, PartitionSpec("core"))
        self.jitted = jax.jit(
            shard_map(_body, mesh=self.mesh, in_specs=in_specs,
                      out_specs=out_specs, check_rep=False),
            donate_argnums=donate, keep_unused=True,
        )
        import jax.numpy as jnp
        zs = list(zero_shapes)

        def _mk_zeros():
            return tuple(jnp.zeros((n_cores * s[0], *s[1:]), d) for s, d in zs)
        self.zeros_fn = jax.jit(_mk_zeros,
                                out_shardings=(self.sharding,) * n_outs)

    def put_shards(self, pool, per_core_arrays):
        jax = self.jax

        def p(c):
            return jax.device_put(
                np.ascontiguousarray(per_core_arrays[c]), self.devices[c])
        return [pool.submit(p, c) for c in range(self.n_cores)]

    def assemble(self, shard_futs):
        jax = self.jax
        shards = [f.result() for f in shard_futs]
        s0 = shards[0].shape
        global_shape = (self.n_cores * s0[0], *s0[1:])
        return jax.make_array_from_single_device_arrays(
            global_shape, self.sharding, shards)

    def put_inputs(self, per_core):
        from concurrent.futures import ThreadPoolExecutor
        with ThreadPoolExecutor(16) as pool:
            futs = {name: self.put_shards(pool, [pc[name] for pc in per_core])
                    for name in self.in_names}
            return [self.assemble(futs[name]) for name in self.in_names]

    def run(self, in_arrays):
        zeros = getattr(self, "_zeros_cache", None)
        self._zeros_cache = None
        if zeros is None:
            zeros = self.zeros_fn()
        return self.jitted(*in_arrays, *zeros)

    def fetch(self, outs):
        from concurrent.futures import ThreadPoolExecutor
        results = []
        for o in outs:
            shards = sorted(o.addressable_shards,
                            key=lambda s: s.index[0].start or 0)
            datas = [None] * len(shards)

            def g(i):
                datas[i] = np.asarray(shards[i].data)
            with ThreadPoolExecutor(len(shards)) as ex:
                list(ex.map(g, range(len(shards))))
            results.append(np.concatenate(datas, axis=0))
        return results

    def warmup(self):
        per_core = []
        for c in range(self.n_cores):
            d = {}
            for nm in self.in_names:
                shape, dt = self._in_shapes[nm]
                d[nm] = np.zeros(shape, dt)
            per_core.append(d)
        arrs = self.put_inputs(per_core)
        outs = self.run(arrs)
        self.jax.block_until_ready(outs)
        self._zeros_cache = self.zeros_fn()
        self.jax.block_until_ready(self._zeros_cache)
        return outs


_hw_init()
